# revision 14
# baseline (speedup 1.0000x reference)
"""Trainium2 Bass kernel for nn_GCNLayer (3-layer GCN + max/mean pooling, T temporal slices).

Self-contained: hardcodes the problem shapes (N=50000, E=800000, B=250, T=8,
CIN=32, COUT=64) and distributes over 8 NeuronCores by graph/dst-node range.

Algorithm per layer (S = sym-normalized adjacency incl. self-loops):
    H_out = relu((S @ H_in) @ W + b)
computed edge-parallel per core:
  - dma_gather of H_in[src] rows (bf16, T-packed rows), round-robined across
    the 4 SWDGE queues so descriptor generation runs on all 4 Q7 core pairs
  - scatter-add via one-hot matmul: lhsT = O (128 edges x 128 dst slots,
    norm values baked in), rhs = gathered messages, PSUM-accumulated per
    128-node dst block
  - PE transpose (bf16) -> fused block-diagonal W matmul -> relu+bias on ACT
    written straight into the pooling group tile
  - pooling (max + mean over each graph's 200 nodes) via free-dim reduces
  - transpose back, store bf16 H to DRAM; AllGather across the 8 cores in
    4 segment chunks so most of the collective overlaps compute
"""

import os
import numpy as np
import ml_dtypes

import concourse.bass as bass
import concourse.mybir as mybir
from concourse import bacc, tile
from concourse.bass_utils import run_bass_kernel_spmd

F32 = mybir.dt.float32
BF16 = mybir.dt.bfloat16
I16 = mybir.dt.int16
P = 128
NQUEUES = 4
SEG_BLOCKS = [16, 16, 8, 8, 2]      # 50 blocks split into AllGather segments
GRP = 4                              # blocks per processing group


class Cfg:
    def __init__(self, N=50000, E=800000, B=250, T=8, CIN=32, COUT=64,
                 NCORES=8, GRAPH=200):
        self.N, self.E, self.B, self.T = N, E, B, T
        self.CIN, self.COUT, self.NCORES, self.GRAPH = CIN, COUT, NCORES, GRAPH
        base, rem = divmod(B, NCORES)
        self.gpc = [base + (1 if c < rem else 0) for c in range(NCORES)]
        self.GPC = max(self.gpc)                      # uniform per-core graph slots
        self.NPC = self.GPC * GRAPH                   # padded nodes per core
        assert self.NPC % P == 0
        self.NBLK = self.NPC // P                     # dst blocks per core
        assert sum(SEG_BLOCKS) == self.NBLK
        self.NPAD = self.NPC * NCORES                 # padded global node count
        # segment row ranges (local, per core) and global bases
        self.seg_rows = [b * P for b in SEG_BLOCKS]
        self.seg_lo_local = np.concatenate([[0], np.cumsum(self.seg_rows)]).astype(np.int64)
        self.seg_lo_global = self.seg_lo_local * NCORES
        self.HALF = int((self.seg_rows[0] + self.seg_rows[1]) * NCORES)
        assert self.HALF <= 32768
        assert self.NPAD - self.HALF <= 32768
        self.CH1 = CIN * T                            # layer-1 feature row
        self.CH = COUT * T                            # layer-2/3 feature row
        assert self.CH % P == 0
        self.NS = self.CH // P                        # psi partition tiles (t-pairs)
        # graph id offset per core
        self.goff = np.concatenate([[0], np.cumsum(self.gpc)]).astype(np.int64)

    def local2seg(self, local):
        """core-local padded row -> segment index"""
        return np.searchsorted(self.seg_lo_local, local, side="right") - 1


def _groups():
    """group block ranges; groups never span AllGather segments."""
    groups = []
    seg_last_group = []
    b0 = 0
    for sb in SEG_BLOCKS:
        end = b0 + sb
        while b0 < end:
            groups.append(list(range(b0, min(b0 + GRP, end))))
            b0 += GRP if b0 + GRP <= end else (end - b0)
        seg_last_group.append(len(groups) - 1)
    return groups, seg_last_group


def preprocess(cfg, x, edge_index, batch, W1, b1, W2, b2, W3, b3):
    """Build all per-core device inputs. Returns (common_inputs, per_core_inputs, meta)."""
    N, E, T, CIN, COUT = cfg.N, cfg.E, cfg.T, cfg.CIN, cfg.COUT
    src = np.asarray(edge_index[0], np.int64)
    dst = np.asarray(edge_index[1], np.int64)

    # degrees incl self-loops, matching the reference
    deg = np.bincount(dst, minlength=N).astype(np.float32) + 1.0
    dinv = (1.0 / np.sqrt(deg)).astype(np.float32)

    batch = np.asarray(batch, np.int64)
    g2c = np.zeros(cfg.B, np.int64)
    for c in range(cfg.NCORES):
        g2c[cfg.goff[c]:cfg.goff[c + 1]] = c
    node_core = g2c[batch]
    first_node_of_core = np.array([cfg.goff[c] * cfg.GRAPH for c in range(cfg.NCORES)],
                                  np.int64)
    local_n = np.arange(N) - first_node_of_core[node_core]
    # padded id: segment-major, then core-major within segment
    seg = cfg.local2seg(local_n)
    seg_rows = np.asarray(cfg.seg_rows, np.int64)
    pad_id = (cfg.seg_lo_global[seg] + node_core * seg_rows[seg]
              + (local_n - cfg.seg_lo_local[seg]))

    srcp = pad_id[src]
    dstc = node_core[dst]
    dstl = local_n[dst]

    # X permuted to [NPAD, T*CIN] (t-major rows), bf16
    Xp = np.zeros((cfg.NPAD, cfg.CH1), np.float32)
    xm = np.moveaxis(np.asarray(x, np.float32), 2, 1).reshape(N, T * CIN)
    Xp[pad_id] = xm

    groups, seg_last_group = _groups()

    # per-core edge bucketing (self-loops included as edges)
    per_core = []
    # per-(block, half) chunk count: max over cores (keeps SPMD program
    # uniform while dropping the global-max padding)
    KT = np.ones((cfg.NBLK, 2), np.int64)
    for c in range(cfg.NCORES):
        m = dstc == c
        es, ed = srcp[m], dstl[m]
        nv = dinv[src[m]] * dinv[dst[m]]
        ln = np.where(node_core == np.int64(c))[0]
        es = np.concatenate([es, pad_id[ln]])
        ed = np.concatenate([ed, local_n[ln]])
        nv = np.concatenate([nv, dinv[ln] * dinv[ln]])
        blk = ed // P
        half = (es >= cfg.HALF).astype(np.int64)
        order = np.lexsort((es, half, blk))
        es, ed, nv, blk, half = (a[order] for a in (es, ed, nv, blk, half))
        per_core.append((es, ed, nv, blk, half))
        for b in range(cfg.NBLK):
            mb = blk == b
            nlo = int(((half == 0) & mb).sum())
            nhi = int(((half == 1) & mb).sum())
            KT[b, 0] = max(KT[b, 0], -(-nlo // P))
            KT[b, 1] = max(KT[b, 1], -(-nhi // P))
    KL, KH = int(KT[:, 0].max()), int(KT[:, 1].max())

    def call_splits(nch):
        out, pos = [], 0
        while pos < nch:
            k = min(8, nch - pos)
            out.append((pos, k))
            pos += k
        return out

    # call table: (half, group_idx, chunk0_in_group, nchunks)
    calls = []
    for h in (0, 1):
        for gi, blks in enumerate(groups):
            nch = int(sum(KT[b, h] for b in blks))
            for pos, k in call_splits(nch):
                calls.append((h, gi, pos, k))
    NCALLS = len(calls)

    chunk_map = {}
    for ci, (h, gi, pos, k) in enumerate(calls):
        for j in range(k):
            chunk_map[(h, gi, pos + j)] = (ci, j)

    # chunk index within a group: chunk q of block b (half h) sits at
    # offset sum(KT[b', h] for earlier blocks in the group) + q
    def chunk_in_group(h, blks, bi, q):
        return int(sum(KT[b, h] for b in blks[:bi]) + q)

    # build per-core O (bf16) + gather idx
    per_core_inputs = []
    for c in range(cfg.NCORES):
        es, ed, nv, blk, half = per_core[c]
        O = np.zeros((NCALLS, P, 8 * P), np.float32)
        idx = np.zeros((P, NCALLS * 64), np.int16)
        for gi, blks in enumerate(groups):
            for h in (0, 1):
                for bi, b in enumerate(blks):
                    K = int(KT[b, h])
                    m = (blk == b) & (half == h)
                    e_s, e_d, e_n = es[m], ed[m], nv[m]
                    n_e = len(e_s)
                    assert n_e <= K * P
                    for k in range(K):
                        ci, j = chunk_map[(h, gi, chunk_in_group(h, blks, bi, k))]
                        lo, hi = k * P, min((k + 1) * P, n_e)
                        cnt = max(0, hi - lo)
                        vals = np.zeros(P, np.int64)
                        if cnt:
                            vals[:cnt] = e_s[lo:hi] - (cfg.HALF if h else 0)
                        ii = j * P + np.arange(P)
                        idx[ii % 16, ci * 64 + ii // 16] = vals.astype(np.int16)
                        if cnt:
                            rows = np.arange(cnt)
                            cols = j * P + (e_d[lo:hi] - b * P)
                            O[ci, rows, cols] = e_n[lo:hi]
        idx[16:] = np.tile(idx[:16], (7, 1))
        per_core_inputs.append({
            "o23": O.astype(ml_dtypes.bfloat16),
            "gidx": idx,
        })

    # pooling piece table per group
    pool_pieces = []
    seen = set()
    for gi, blks in enumerate(groups):
        n0g = blks[0] * P
        n1g = (blks[-1] + 1) * P
        pieces = []
        n = n0g
        while n < n1g:
            gl = n // cfg.GRAPH
            nend = min((gl + 1) * cfg.GRAPH, n1g)
            ft = gl not in seen
            seen.add(gl)
            pieces.append((n - n0g, nend - n0g, gl, ft))
            n = nend
        pool_pieces.append(pieces)

    # fused block-diagonal psi weights: [4, 128, 128]
    #   0: layer0 variant A (q=0,1 -> s_out even), 1: layer0 variant B (q=2,3)
    #   2: layer1, 3: layer2
    wz = np.zeros((4, P, P), np.float32)
    W1, W2, W3 = (np.asarray(w, np.float32) for w in (W1, W2, W3))
    wz[0, 0:32, 0:64] = W1
    wz[0, 32:64, 64:128] = W1
    wz[1, 64:96, 0:64] = W1
    wz[1, 96:128, 64:128] = W1
    wz[2, 0:64, 0:64] = W2
    wz[2, 64:128, 64:128] = W2
    wz[3, 0:64, 0:64] = W3
    wz[3, 64:128, 64:128] = W3

    bias_col = np.zeros((P, 3), np.float32)
    for i, b in enumerate((b1, b2, b3)):
        bias_col[:, i] = np.tile(np.asarray(b, np.float32), P // COUT)

    ident = np.eye(P, dtype=np.float32)
    common = {
        "xp": Xp.astype(ml_dtypes.bfloat16),
        "wz": wz.astype(ml_dtypes.bfloat16),
        "biascol": bias_col,
        "id_bf": ident.astype(ml_dtypes.bfloat16),
    }
    meta = dict(KL=KL, KH=KH, KT=KT, calls=calls, chunk_map=chunk_map,
                groups=groups, seg_last_group=seg_last_group,
                pool_pieces=pool_pieces, NCALLS=NCALLS)
    return common, per_core_inputs, meta


def build(cfg, meta):
    """Construct the Bass/Tile SPMD program."""
    KL, KH, calls, chunk_map = meta["KL"], meta["KH"], meta["calls"], meta["chunk_map"]
    KT = meta["KT"]

    def chunk_in_group(h, blks, bi, q):
        return int(sum(KT[b, h] for b in blks[:bi]) + q)
    groups, pool_pieces, NCALLS = meta["groups"], meta["pool_pieces"], meta["NCALLS"]
    seg_last_group = meta["seg_last_group"]
    NS, CH, CH1, T, COUT = cfg.NS, cfg.CH, cfg.CH1, cfg.T, cfg.COUT
    NS1 = max(CH1 // P, 1)
    CIN = cfg.CIN

    nc = bacc.Bacc("TRN2", target_bir_lowering=False, debug=False,
                   num_devices=cfg.NCORES, num_swdge_queues=NQUEUES)

    xp = nc.dram_tensor("xp", [cfg.NPAD, CH1], BF16, kind="ExternalInput")
    o23 = nc.dram_tensor("o23", [NCALLS, P, 8 * P], BF16, kind="ExternalInput")
    gidx = nc.dram_tensor("gidx", [P, NCALLS * 64], I16, kind="ExternalInput")
    wz_d = nc.dram_tensor("wz", [4, P, P], BF16, kind="ExternalInput")
    biascol = nc.dram_tensor("biascol", [P, 3], F32, kind="ExternalInput")
    id_bf = nc.dram_tensor("id_bf", [P, P], BF16, kind="ExternalInput")
    out = nc.dram_tensor("out", [P, 2 * NS * cfg.GPC], F32, kind="ExternalOutput")

    rg = [list(range(cfg.NCORES))]
    qn = [0]  # round-robin SWDGE queue counter

    with tile.TileContext(nc) as tc:
        with (
            tc.tile_pool(name="const", bufs=1) as constp,
            tc.tile_pool(name="msg", bufs=9) as msgp,
            tc.tile_pool(name="msgh", bufs=6) as msghp,
            tc.tile_pool(name="otile", bufs=6) as otp,
            tc.tile_pool(name="oth", bufs=6) as othp,
            tc.tile_pool(name="work", bufs=4) as workp,
            tc.tile_pool(name="psig", bufs=2) as psigp,
            tc.tile_pool(name="pool", bufs=1) as poolp,
            tc.tile_pool(name="gps", bufs=3, space="PSUM") as gpsp,
            tc.tile_pool(name="t1ps", bufs=2, space="PSUM") as t1psp,
            tc.tile_pool(name="psips", bufs=2, space="PSUM") as psipsp,
            tc.tile_pool(name="t2ps", bufs=1, space="PSUM") as t2psp,
            tc.tile_pool(name="dram", bufs=1, space="DRAM") as dramp,
        ):
            # ---- constants into SBUF
            idx_sb = constp.tile([P, NCALLS * 64], I16)
            nc.sync.dma_start(out=idx_sb[:], in_=gidx[:])
            wzt = constp.tile([P, 4 * P], BF16, tag="wzt")
            nc.sync.dma_start(
                out=wzt[:].rearrange("p (i m) -> p i m", i=4),
                in_=wz_d.ap().rearrange("i p m -> p i m"))
            bct = constp.tile([P, 3], F32)
            nc.sync.dma_start(out=bct[:], in_=biascol[:])
            idb = constp.tile([P, P], BF16)
            nc.sync.dma_start(out=idb[:], in_=id_bf[:])

            # ---- pool accumulators
            lmax = poolp.tile([P, NS * cfg.GPC], F32, tag="lmax")
            lsum = poolp.tile([P, NS * cfg.GPC], F32, tag="lsum")
            fmax = poolp.tile([P, NS * cfg.GPC], F32, tag="fmax")
            fsum = poolp.tile([P, NS * cfg.GPC], F32, tag="fsum")
            for _t in (lmax, lsum, fmax, fsum):
                nc.vector.memset(_t[:], 0.0)

            # ---- DRAM intermediates
            h_mine = []
            h_full = []
            for i in range(2):
                hm = dramp.tile([cfg.NPC, CH], BF16, tag=f"hm{i}")
                h_mine.append(hm)
                hf = dramp.tile([cfg.NPAD, CH], BF16, tag=f"hf{i}")
                h_full.append(hf)

            nlayers = int(os.environ.get("GCN_LAYERS", "3"))
            pending_ags = []

            def emit_due_ags(li, gi):
                for ent in list(pending_ags):
                    (dli, dgi), ali, si = ent
                    if (dli, dgi) != (li, gi):
                        continue
                    pending_ags.remove(ent)
                    llo = int(cfg.seg_lo_local[si])
                    lhi = int(cfg.seg_lo_local[si + 1])
                    glo = int(cfg.seg_lo_global[si])
                    ghi = int(cfg.seg_lo_global[si + 1])
                    nc.gpsimd.collective_compute(
                        "AllGather", mybir.AluOpType.bypass,
                        replica_groups=rg,
                        ins=[h_mine[ali][llo:lhi, :]],
                        outs=[h_full[ali][glo:ghi, :]],
                    )

            def layer(li):
                ch_in = CH1 if li == 0 else CH
                ns_in = NS1 if li == 0 else NS
                if li == 0:
                    hsrc = xp
                else:
                    hsrc = h_full[li - 1]
                src_lo = hsrc[:cfg.HALF, :]
                src_hi = hsrc[cfg.HALF:cfg.NPAD, :]

                lo_calls, hi_calls = {}, {}
                for ci, (h, gi, pos, k) in enumerate(calls):
                    (lo_calls if h == 0 else hi_calls).setdefault(gi, []).append(
                        (ci, h, pos, k))

                gtiles = {}

                def emit_calls(cl):
                    for ci, h, pos, k in cl:
                        ni = k * P
                        g = (msgp if h == 0 else msghp).tile(
                            [P, 8 * ch_in], BF16, tag=f"m{h}")
                        nc.gpsimd.dma_gather(
                            out_ap=g[:, :k * ch_in].rearrange(
                                "p (c e) -> p c e", e=ch_in),
                            in_ap=(src_lo if h == 0 else src_hi),
                            idxs_ap=idx_sb[:, ci * 64: ci * 64 + max(ni // 16, 1)],
                            num_idxs=ni,
                            num_idxs_reg=ni,
                            elem_size=ch_in,
                            queue_num=qn[0] % NQUEUES,
                        )
                        qn[0] += 1
                        ot = (otp if h == 0 else othp).tile(
                            [P, 8 * P], BF16, tag=f"oo{h}")
                        nc.sync.dma_start(out=ot[:, :k * P], in_=o23[ci, :, :k * P])
                        gtiles[ci] = (g, ot)

                ngroups = len(groups)
                for gi, blks in enumerate(groups):
                    # lo-half gathers run one group ahead; hi-half gathers of
                    # this group go last so a pending tail AllGather (high
                    # segments) doesn't head-of-line-block the stream.
                    if gi == 0:
                        emit_calls(lo_calls[0])
                    if gi + 1 < ngroups:
                        emit_calls(lo_calls[gi + 1])
                    emit_due_ags(li, gi)
                    emit_calls(hi_calls[gi])

                    psi_grp = psigp.tile([P, NS * len(blks) * P], F32, tag="psig")
                    # ---- phase 1: scatter matmuls + PSUM->SBUF bf16 cast
                    gbfbs = {}
                    for bi, b in enumerate(blks):
                        gps = gpsp.tile([P, ch_in], F32, tag="gps")
                        nmm = int(KT[b, 0] + KT[b, 1])
                        mm = 0
                        for h in (0, 1):
                            K = int(KT[b, h])
                            for k in range(K):
                                ci, j = chunk_map[(h, gi, chunk_in_group(h, blks, bi, k))]
                                g, ot = gtiles[ci]
                                nc.tensor.matmul(
                                    gps[:],
                                    lhsT=ot[:, j * P:(j + 1) * P],
                                    rhs=g[:, j * ch_in:(j + 1) * ch_in],
                                    start=(mm == 0), stop=(mm == nmm - 1),
                                )
                                mm += 1
                        gbfb = workp.tile([P, ch_in], BF16, tag="gbfb")
                        nc.scalar.activation(
                            gbfb[:], gps[:], mybir.ActivationFunctionType.Copy)
                        gbfbs[bi] = gbfb
                    # ---- phase 2: transpose to channel-major
                    gts = {}
                    for bi, b in enumerate(blks):
                        t1 = t1psp.tile([P, ns_in * P], BF16, tag="t1")
                        for s in range(ns_in):
                            nc.tensor.transpose(
                                t1[:, s * P:(s + 1) * P],
                                gbfbs[bi][:, s * P:(s + 1) * P], idb[:])
                        gt = workp.tile([P, ns_in * P], BF16, tag="gt")
                        nc.vector.tensor_copy(out=gt[:], in_=t1[:])
                        gts[bi] = gt
                    # ---- phase 3: W matmul + relu into pool tile + h store
                    if "psi" in os.environ.get("GCN_SKIP", ""):
                        continue
                    for bi, b in enumerate(blks):
                        gt = gts[bi]
                        psi_ps = psipsp.tile([P, NS * P], F32, tag="psip")
                        if li == 0:
                            for s_in in range(NS1):
                                for v in range(2):
                                    nc.tensor.matmul(
                                        psi_ps[:, (2 * s_in + v) * P:
                                               (2 * s_in + v + 1) * P],
                                        lhsT=wzt[:, v * P:(v + 1) * P],
                                        rhs=gt[:, s_in * P:(s_in + 1) * P],
                                        start=True, stop=True,
                                    )
                        else:
                            wv = 1 + li
                            for s in range(NS):
                                nc.tensor.matmul(
                                    psi_ps[:, s * P:(s + 1) * P],
                                    lhsT=wzt[:, wv * P:(wv + 1) * P],
                                    rhs=gt[:, s * P:(s + 1) * P],
                                    start=True, stop=True,
                                )
                        # relu + bias straight into the group pooling tile
                        gwk = len(blks) * P
                        dst_view = psi_grp[:].rearrange(
                            "p (s n) -> p s n", n=gwk)[:, :, bi * P:(bi + 1) * P]
                        nc.scalar.activation(
                            dst_view,
                            psi_ps[:].rearrange("p (s n) -> p s n", s=NS),
                            mybir.ActivationFunctionType.Relu,
                            bias=bct[:, li:li + 1],
                        )
                        if li < 2 and nlayers > li + 1 and \
                                "t2" not in os.environ.get("GCN_SKIP", ""):
                            psb = workp.tile([P, NS * P], BF16, tag="psb")
                            nc.scalar.activation(
                                psb[:], psi_ps[:],
                                mybir.ActivationFunctionType.Relu,
                                bias=bct[:, li:li + 1],
                            )
                            t2 = t2psp.tile([P, NS * P], BF16, tag="t2")
                            for s in range(NS):
                                nc.tensor.transpose(
                                    t2[:, s * P:(s + 1) * P],
                                    psb[:, s * P:(s + 1) * P], idb[:])
                            hbf = workp.tile([P, CH], BF16, tag="hbf")
                            nc.vector.tensor_copy(out=hbf[:], in_=t2[:])
                            nc.sync.dma_start(
                                out=h_mine[li][b * P:(b + 1) * P, :], in_=hbf[:])

                    # ---- pooling for this group
                    if "pool" not in os.environ.get("GCN_SKIP", ""):
                        gw = len(blks) * P
                        for s in range(NS):
                            base = s * gw
                            for (n0, n1, gl, ft) in pool_pieces[gi]:
                                seg = psi_grp[:, base + n0: base + n1]
                                slot = slice(s * cfg.GPC + gl, s * cfg.GPC + gl + 1)
                                if ft:
                                    nc.vector.reduce_max(
                                        out=lmax[:, slot], in_=seg,
                                        axis=mybir.AxisListType.X)
                                    nc.vector.reduce_sum(
                                        out=lsum[:, slot], in_=seg,
                                        axis=mybir.AxisListType.X)
                                else:
                                    tm = workp.tile([P, 2], F32, tag="ptmp")
                                    nc.vector.reduce_max(out=tm[:, 0:1], in_=seg,
                                                         axis=mybir.AxisListType.X)
                                    nc.vector.reduce_sum(out=tm[:, 1:2], in_=seg,
                                                         axis=mybir.AxisListType.X)
                                    nc.vector.tensor_tensor(
                                        out=lmax[:, slot], in0=lmax[:, slot],
                                        in1=tm[:, 0:1], op=mybir.AluOpType.max)
                                    nc.vector.tensor_add(
                                        out=lsum[:, slot], in0=lsum[:, slot],
                                        in1=tm[:, 1:2])

                    # ---- AllGather: schedule per completed segment, but
                    # emit the trigger ~2 groups later so its h_mine deps are
                    # already satisfied and it never stalls the gather stream
                    if li < 2 and nlayers > li + 1 and gi in seg_last_group:
                        si = seg_last_group.index(gi)
                        dgi = min(gi + 2, ngroups - 1)
                        due = (li, dgi) if dgi > gi else (li + 1, 0)
                        pending_ags.append((due, li, si))

                # ---- layer end: accumulate pools
                if "pool" in os.environ.get("GCN_SKIP", ""):
                    pass
                elif li == 0:
                    nc.vector.tensor_copy(out=fmax[:], in_=lmax[:])
                    nc.vector.tensor_copy(out=fsum[:], in_=lsum[:])
                else:
                    nc.vector.tensor_add(out=fmax[:], in0=fmax[:], in1=lmax[:])
                    nc.vector.tensor_add(out=fsum[:], in0=fsum[:], in1=lsum[:])

            for _li in range(nlayers):
                layer(_li)

            # mean = sum / GRAPH
            nc.vector.tensor_scalar_mul(fsum[:], fsum[:],
                                        float(np.float32(1.0 / cfg.GRAPH)))
            osb = workp.tile([P, 2 * NS * cfg.GPC], F32, tag="osb")
            nc.vector.tensor_copy(out=osb[:, :NS * cfg.GPC], in_=fmax[:])
            nc.vector.tensor_copy(out=osb[:, NS * cfg.GPC:], in_=fsum[:])
            nc.sync.dma_start(out=out[:], in_=osb[:])

    nc.compile()
    return nc


def unshard(cfg, results):
    """[NCORES][128, 2*NS*GPC] -> [B, 2*COUT, T] float32."""
    B, T, COUT, NS, GPC = cfg.B, cfg.T, cfg.COUT, cfg.NS, cfg.GPC
    out = np.zeros((B, 2 * COUT, T), np.float32)
    for c in range(cfg.NCORES):
        V = results[c]["out"]
        for gl in range(cfg.gpc[c]):
            g = cfg.goff[c] + gl
            for s in range(NS):
                for half in range(2):
                    t_ = 2 * s + half
                    co = np.arange(COUT)
                    pp = half * COUT + co
                    out[g, co, t_] = V[pp, s * GPC + gl]
                    out[g, COUT + co, t_] = V[pp, NS * GPC + s * GPC + gl]
    return out


_CACHE = {}


def kernel(**inputs):
    cfg = Cfg()
    common, per_core, meta = preprocess(
        cfg, inputs["x"], inputs["edge_index"], inputs["batch"],
        inputs["W1"], inputs["b1"], inputs["W2"], inputs["b2"],
        inputs["W3"], inputs["b3"])
    key = (meta["KL"], meta["KH"], meta["NCALLS"], meta["KT"].tobytes())
    if key not in _CACHE:
        _CACHE[key] = build(cfg, meta)
    nc = _CACHE[key]
    in_maps = []
    for c in range(cfg.NCORES):
        m = dict(common)
        m["o23"] = per_core[c]["o23"]
        m["gidx"] = per_core[c]["gidx"]
        in_maps.append(m)
    res = run_bass_kernel_spmd(nc, in_maps, list(range(cfg.NCORES)))
    return unshard(cfg, res.results)


# revision 15
# speedup vs baseline: 1.0558x; 1.0558x over previous
"""Trainium2 Bass kernel for nn_GCNLayer (3-layer GCN + max/mean pooling, T temporal slices).

Self-contained: hardcodes the problem shapes (N=50000, E=800000, B=250, T=8,
CIN=32, COUT=64) and distributes over 8 NeuronCores by graph/dst-node range.

Algorithm per layer (S = sym-normalized adjacency incl. self-loops):
    H_out = relu((S @ H_in) @ W + b)
computed edge-parallel per core:
  - dma_gather of H_in[src] rows (bf16, T-packed rows), round-robined across
    the 4 SWDGE queues so descriptor generation runs on all 4 Q7 core pairs
  - scatter-add via one-hot matmul: lhsT = O (128 edges x 128 dst slots,
    norm values baked in), rhs = gathered messages, PSUM-accumulated per
    128-node dst block
  - PE transpose (bf16) -> fused block-diagonal W matmul -> relu+bias on ACT
    written straight into the pooling group tile
  - pooling (max + mean over each graph's 200 nodes) via free-dim reduces
  - transpose back, store bf16 H to DRAM; AllGather across the 8 cores in
    4 segment chunks so most of the collective overlaps compute
"""

import os
import numpy as np
import ml_dtypes

import concourse.bass as bass
import concourse.mybir as mybir
from concourse import bacc, tile
from concourse.bass_utils import run_bass_kernel_spmd

F32 = mybir.dt.float32
BF16 = mybir.dt.bfloat16
I16 = mybir.dt.int16
P = 128
NQUEUES = 4
SEG_BLOCKS = [16, 16, 8, 8, 2]      # 50 blocks split into AllGather segments
GRP = 4                              # blocks per processing group


class Cfg:
    def __init__(self, N=50000, E=800000, B=250, T=8, CIN=32, COUT=64,
                 NCORES=8, GRAPH=200):
        self.N, self.E, self.B, self.T = N, E, B, T
        self.CIN, self.COUT, self.NCORES, self.GRAPH = CIN, COUT, NCORES, GRAPH
        base, rem = divmod(B, NCORES)
        self.gpc = [base + (1 if c < rem else 0) for c in range(NCORES)]
        self.GPC = max(self.gpc)                      # uniform per-core graph slots
        self.NPC = self.GPC * GRAPH                   # padded nodes per core
        assert self.NPC % P == 0
        self.NBLK = self.NPC // P                     # dst blocks per core
        assert sum(SEG_BLOCKS) == self.NBLK
        self.NPAD = self.NPC * NCORES                 # padded global node count
        # segment row ranges (local, per core) and global bases
        self.seg_rows = [b * P for b in SEG_BLOCKS]
        self.seg_lo_local = np.concatenate([[0], np.cumsum(self.seg_rows)]).astype(np.int64)
        self.seg_lo_global = self.seg_lo_local * NCORES
        self.HALF = int((self.seg_rows[0] + self.seg_rows[1]) * NCORES)
        assert self.HALF <= 32768
        assert self.NPAD - self.HALF <= 32768
        self.CH1 = CIN * T                            # layer-1 feature row
        self.CH = COUT * T                            # layer-2/3 feature row
        assert self.CH % P == 0
        self.NS = self.CH // P                        # psi partition tiles (t-pairs)
        # graph id offset per core
        self.goff = np.concatenate([[0], np.cumsum(self.gpc)]).astype(np.int64)

    def local2seg(self, local):
        """core-local padded row -> segment index"""
        return np.searchsorted(self.seg_lo_local, local, side="right") - 1


def _groups():
    """group block ranges; groups never span AllGather segments."""
    groups = []
    seg_last_group = []
    b0 = 0
    for sb in SEG_BLOCKS:
        end = b0 + sb
        while b0 < end:
            groups.append(list(range(b0, min(b0 + GRP, end))))
            b0 += GRP if b0 + GRP <= end else (end - b0)
        seg_last_group.append(len(groups) - 1)
    return groups, seg_last_group


def preprocess(cfg, x, edge_index, batch, W1, b1, W2, b2, W3, b3):
    """Build all per-core device inputs. Returns (common_inputs, per_core_inputs, meta)."""
    N, E, T, CIN, COUT = cfg.N, cfg.E, cfg.T, cfg.CIN, cfg.COUT
    src = np.asarray(edge_index[0], np.int64)
    dst = np.asarray(edge_index[1], np.int64)

    # degrees incl self-loops, matching the reference
    deg = np.bincount(dst, minlength=N).astype(np.float32) + 1.0
    dinv = (1.0 / np.sqrt(deg)).astype(np.float32)

    batch = np.asarray(batch, np.int64)
    g2c = np.zeros(cfg.B, np.int64)
    for c in range(cfg.NCORES):
        g2c[cfg.goff[c]:cfg.goff[c + 1]] = c
    node_core = g2c[batch]
    first_node_of_core = np.array([cfg.goff[c] * cfg.GRAPH for c in range(cfg.NCORES)],
                                  np.int64)
    local_n = np.arange(N) - first_node_of_core[node_core]
    # padded id: segment-major, then core-major within segment
    seg = cfg.local2seg(local_n)
    seg_rows = np.asarray(cfg.seg_rows, np.int64)
    pad_id = (cfg.seg_lo_global[seg] + node_core * seg_rows[seg]
              + (local_n - cfg.seg_lo_local[seg]))

    srcp = pad_id[src]
    dstc = node_core[dst]
    dstl = local_n[dst]

    # X permuted to [NPAD, T*CIN] (t-major rows), bf16
    Xp = np.zeros((cfg.NPAD, cfg.CH1), np.float32)
    xm = np.moveaxis(np.asarray(x, np.float32), 2, 1).reshape(N, T * CIN)
    Xp[pad_id] = xm

    groups, seg_last_group = _groups()

    # per-core edge bucketing (self-loops included as edges)
    per_core = []
    # per-(block, half) chunk count: max over cores (keeps SPMD program
    # uniform while dropping the global-max padding)
    KT = np.ones((cfg.NBLK, 2), np.int64)
    for c in range(cfg.NCORES):
        m = dstc == c
        es, ed = srcp[m], dstl[m]
        nv = dinv[src[m]] * dinv[dst[m]]
        ln = np.where(node_core == np.int64(c))[0]
        es = np.concatenate([es, pad_id[ln]])
        ed = np.concatenate([ed, local_n[ln]])
        nv = np.concatenate([nv, dinv[ln] * dinv[ln]])
        blk = ed // P
        half = (es >= cfg.HALF).astype(np.int64)
        order = np.lexsort((es, half, blk))
        es, ed, nv, blk, half = (a[order] for a in (es, ed, nv, blk, half))
        per_core.append((es, ed, nv, blk, half))
        for b in range(cfg.NBLK):
            mb = blk == b
            nlo = int(((half == 0) & mb).sum())
            nhi = int(((half == 1) & mb).sum())
            KT[b, 0] = max(KT[b, 0], -(-nlo // P))
            KT[b, 1] = max(KT[b, 1], -(-nhi // P))
    KL, KH = int(KT[:, 0].max()), int(KT[:, 1].max())

    def call_splits(nch):
        out, pos = [], 0
        while pos < nch:
            k = min(8, nch - pos)
            out.append((pos, k))
            pos += k
        return out

    # call table: (half, group_idx, chunk0_in_group, nchunks)
    calls = []
    for h in (0, 1):
        for gi, blks in enumerate(groups):
            nch = int(sum(KT[b, h] for b in blks))
            for pos, k in call_splits(nch):
                calls.append((h, gi, pos, k))
    NCALLS = len(calls)

    chunk_map = {}
    for ci, (h, gi, pos, k) in enumerate(calls):
        for j in range(k):
            chunk_map[(h, gi, pos + j)] = (ci, j)

    # chunk index within a group: chunk q of block b (half h) sits at
    # offset sum(KT[b', h] for earlier blocks in the group) + q
    def chunk_in_group(h, blks, bi, q):
        return int(sum(KT[b, h] for b in blks[:bi]) + q)

    # build per-core O (bf16) + gather idx
    per_core_inputs = []
    for c in range(cfg.NCORES):
        es, ed, nv, blk, half = per_core[c]
        O = np.zeros((NCALLS, P, 8 * P), np.float32)
        idx = np.zeros((P, NCALLS * 64), np.int16)
        for gi, blks in enumerate(groups):
            for h in (0, 1):
                for bi, b in enumerate(blks):
                    K = int(KT[b, h])
                    m = (blk == b) & (half == h)
                    e_s, e_d, e_n = es[m], ed[m], nv[m]
                    n_e = len(e_s)
                    assert n_e <= K * P
                    for k in range(K):
                        ci, j = chunk_map[(h, gi, chunk_in_group(h, blks, bi, k))]
                        lo, hi = k * P, min((k + 1) * P, n_e)
                        cnt = max(0, hi - lo)
                        vals = np.zeros(P, np.int64)
                        if cnt:
                            vals[:cnt] = e_s[lo:hi] - (cfg.HALF if h else 0)
                        ii = j * P + np.arange(P)
                        idx[ii % 16, ci * 64 + ii // 16] = vals.astype(np.int16)
                        if cnt:
                            rows = np.arange(cnt)
                            cols = j * P + (e_d[lo:hi] - b * P)
                            O[ci, rows, cols] = e_n[lo:hi]
        idx[16:] = np.tile(idx[:16], (7, 1))
        per_core_inputs.append({
            "o23": O.astype(ml_dtypes.bfloat16),
            "gidx": idx,
        })

    # pooling piece table per group
    pool_pieces = []
    seen = set()
    for gi, blks in enumerate(groups):
        n0g = blks[0] * P
        n1g = (blks[-1] + 1) * P
        pieces = []
        n = n0g
        while n < n1g:
            gl = n // cfg.GRAPH
            nend = min((gl + 1) * cfg.GRAPH, n1g)
            ft = gl not in seen
            seen.add(gl)
            pieces.append((n - n0g, nend - n0g, gl, ft))
            n = nend
        pool_pieces.append(pieces)

    # fused block-diagonal psi weights: [4, 128, 128]
    #   0: layer0 variant A (q=0,1 -> s_out even), 1: layer0 variant B (q=2,3)
    #   2: layer1, 3: layer2
    wz = np.zeros((4, P, P), np.float32)
    W1, W2, W3 = (np.asarray(w, np.float32) for w in (W1, W2, W3))
    wz[0, 0:32, 0:64] = W1
    wz[0, 32:64, 64:128] = W1
    wz[1, 64:96, 0:64] = W1
    wz[1, 96:128, 64:128] = W1
    wz[2, 0:64, 0:64] = W2
    wz[2, 64:128, 64:128] = W2
    wz[3, 0:64, 0:64] = W3
    wz[3, 64:128, 64:128] = W3

    bias_col = np.zeros((P, 3), np.float32)
    for i, b in enumerate((b1, b2, b3)):
        bias_col[:, i] = np.tile(np.asarray(b, np.float32), P // COUT)

    ident = np.eye(P, dtype=np.float32)
    common = {
        "xp": Xp.astype(ml_dtypes.bfloat16),
        "wz": wz.astype(ml_dtypes.bfloat16),
        "biascol": bias_col,
        "id_bf": ident.astype(ml_dtypes.bfloat16),
    }
    meta = dict(KL=KL, KH=KH, KT=KT, calls=calls, chunk_map=chunk_map,
                groups=groups, seg_last_group=seg_last_group,
                pool_pieces=pool_pieces, NCALLS=NCALLS)
    return common, per_core_inputs, meta


def build(cfg, meta):
    """Construct the Bass/Tile SPMD program."""
    KL, KH, calls, chunk_map = meta["KL"], meta["KH"], meta["calls"], meta["chunk_map"]
    KT = meta["KT"]

    def chunk_in_group(h, blks, bi, q):
        return int(sum(KT[b, h] for b in blks[:bi]) + q)
    groups, pool_pieces, NCALLS = meta["groups"], meta["pool_pieces"], meta["NCALLS"]
    seg_last_group = meta["seg_last_group"]
    NS, CH, CH1, T, COUT = cfg.NS, cfg.CH, cfg.CH1, cfg.T, cfg.COUT
    NS1 = max(CH1 // P, 1)
    CIN = cfg.CIN

    nc = bacc.Bacc("TRN2", target_bir_lowering=False, debug=False,
                   num_devices=cfg.NCORES, num_swdge_queues=NQUEUES)

    xp = nc.dram_tensor("xp", [cfg.NPAD, CH1], BF16, kind="ExternalInput")
    o23 = nc.dram_tensor("o23", [NCALLS, P, 8 * P], BF16, kind="ExternalInput")
    gidx = nc.dram_tensor("gidx", [P, NCALLS * 64], I16, kind="ExternalInput")
    wz_d = nc.dram_tensor("wz", [4, P, P], BF16, kind="ExternalInput")
    biascol = nc.dram_tensor("biascol", [P, 3], F32, kind="ExternalInput")
    id_bf = nc.dram_tensor("id_bf", [P, P], BF16, kind="ExternalInput")
    out = nc.dram_tensor("out", [P, 2 * NS * cfg.GPC], F32, kind="ExternalOutput")

    rg = [list(range(cfg.NCORES))]
    qn = [0]  # round-robin SWDGE queue counter

    with tile.TileContext(nc) as tc:
        with (
            tc.tile_pool(name="const", bufs=1) as constp,
            tc.tile_pool(name="msg", bufs=10) as msgp,
            tc.tile_pool(name="msgh", bufs=5) as msghp,
            tc.tile_pool(name="otile", bufs=6) as otp,
            tc.tile_pool(name="oth", bufs=5) as othp,
            tc.tile_pool(name="work", bufs=4) as workp,
            tc.tile_pool(name="psig", bufs=2) as psigp,
            tc.tile_pool(name="pool", bufs=1) as poolp,
            tc.tile_pool(name="gps", bufs=3, space="PSUM") as gpsp,
            tc.tile_pool(name="t1ps", bufs=2, space="PSUM") as t1psp,
            tc.tile_pool(name="psips", bufs=2, space="PSUM") as psipsp,
            tc.tile_pool(name="t2ps", bufs=1, space="PSUM") as t2psp,
            tc.tile_pool(name="dram", bufs=1, space="DRAM") as dramp,
        ):
            # ---- constants into SBUF
            idx_sb = constp.tile([P, NCALLS * 64], I16)
            nc.sync.dma_start(out=idx_sb[:], in_=gidx[:])
            wzt = constp.tile([P, 4 * P], BF16, tag="wzt")
            nc.sync.dma_start(
                out=wzt[:].rearrange("p (i m) -> p i m", i=4),
                in_=wz_d.ap().rearrange("i p m -> p i m"))
            bct = constp.tile([P, 3], F32)
            nc.sync.dma_start(out=bct[:], in_=biascol[:])
            idb = constp.tile([P, P], BF16)
            nc.sync.dma_start(out=idb[:], in_=id_bf[:])

            # ---- pool accumulators
            lmax = poolp.tile([P, NS * cfg.GPC], F32, tag="lmax")
            lsum = poolp.tile([P, NS * cfg.GPC], F32, tag="lsum")
            fmax = poolp.tile([P, NS * cfg.GPC], F32, tag="fmax")
            fsum = poolp.tile([P, NS * cfg.GPC], F32, tag="fsum")
            for _t in (lmax, lsum, fmax, fsum):
                nc.vector.memset(_t[:], 0.0)

            # ---- DRAM intermediates
            h_mine = []
            h_full = []
            for i in range(2):
                hm = dramp.tile([cfg.NPC, CH], BF16, tag=f"hm{i}")
                h_mine.append(hm)
                hf = dramp.tile([cfg.NPAD, CH], BF16, tag=f"hf{i}")
                h_full.append(hf)

            nlayers = int(os.environ.get("GCN_LAYERS", "3"))
            pending_ags = []

            def emit_due_ags(li, gi):
                for ent in list(pending_ags):
                    (dli, dgi), ali, si = ent
                    if (dli, dgi) != (li, gi):
                        continue
                    pending_ags.remove(ent)
                    llo = int(cfg.seg_lo_local[si])
                    lhi = int(cfg.seg_lo_local[si + 1])
                    glo = int(cfg.seg_lo_global[si])
                    ghi = int(cfg.seg_lo_global[si + 1])
                    nc.gpsimd.collective_compute(
                        "AllGather", mybir.AluOpType.bypass,
                        replica_groups=rg,
                        ins=[h_mine[ali][llo:lhi, :]],
                        outs=[h_full[ali][glo:ghi, :]],
                    )

            def layer(li):
                ch_in = CH1 if li == 0 else CH
                ns_in = NS1 if li == 0 else NS
                if li == 0:
                    hsrc = xp
                else:
                    hsrc = h_full[li - 1]
                src_lo = hsrc[:cfg.HALF, :]
                src_hi = hsrc[cfg.HALF:cfg.NPAD, :]

                lo_calls, hi_calls = {}, {}
                for ci, (h, gi, pos, k) in enumerate(calls):
                    (lo_calls if h == 0 else hi_calls).setdefault(gi, []).append(
                        (ci, h, pos, k))

                gtiles = {}

                def emit_calls(cl):
                    for ci, h, pos, k in cl:
                        ni = k * P
                        g = (msgp if h == 0 else msghp).tile(
                            [P, 8 * ch_in], BF16, tag=f"m{h}")
                        nc.gpsimd.dma_gather(
                            out_ap=g[:, :k * ch_in].rearrange(
                                "p (c e) -> p c e", e=ch_in),
                            in_ap=(src_lo if h == 0 else src_hi),
                            idxs_ap=idx_sb[:, ci * 64: ci * 64 + max(ni // 16, 1)],
                            num_idxs=ni,
                            num_idxs_reg=ni,
                            elem_size=ch_in,
                            queue_num=qn[0] % NQUEUES,
                        )
                        qn[0] += 1
                        ot = (otp if h == 0 else othp).tile(
                            [P, 8 * P], BF16, tag=f"oo{h}")
                        nc.sync.dma_start(out=ot[:, :k * P], in_=o23[ci, :, :k * P])
                        gtiles[ci] = (g, ot)

                ngroups = len(groups)
                for gi, blks in enumerate(groups):
                    # lo-half gathers run one group ahead; hi-half gathers of
                    # this group go last so a pending tail AllGather (high
                    # segments) doesn't head-of-line-block the stream.
                    if gi == 0:
                        emit_calls(lo_calls[0])
                    if gi + 1 < ngroups:
                        emit_calls(lo_calls[gi + 1])
                    emit_due_ags(li, gi)
                    emit_calls(hi_calls[gi])

                    psi_grp = psigp.tile([P, NS * len(blks) * P], F32, tag="psig")
                    # ---- phase 1: scatter matmuls + PSUM->SBUF bf16 cast
                    gbfbs = {}
                    for bi, b in enumerate(blks):
                        gps = gpsp.tile([P, ch_in], F32, tag="gps")
                        nmm = int(KT[b, 0] + KT[b, 1])
                        mm = 0
                        for h in (0, 1):
                            K = int(KT[b, h])
                            for k in range(K):
                                ci, j = chunk_map[(h, gi, chunk_in_group(h, blks, bi, k))]
                                g, ot = gtiles[ci]
                                nc.tensor.matmul(
                                    gps[:],
                                    lhsT=ot[:, j * P:(j + 1) * P],
                                    rhs=g[:, j * ch_in:(j + 1) * ch_in],
                                    start=(mm == 0), stop=(mm == nmm - 1),
                                )
                                mm += 1
                        gbfb = workp.tile([P, ch_in], BF16, tag="gbfb")
                        nc.scalar.activation(
                            gbfb[:], gps[:], mybir.ActivationFunctionType.Copy)
                        gbfbs[bi] = gbfb
                    # ---- phase 2: transpose to channel-major
                    gts = {}
                    for bi, b in enumerate(blks):
                        t1 = t1psp.tile([P, ns_in * P], BF16, tag="t1")
                        for s in range(ns_in):
                            nc.tensor.transpose(
                                t1[:, s * P:(s + 1) * P],
                                gbfbs[bi][:, s * P:(s + 1) * P], idb[:])
                        gt = workp.tile([P, ns_in * P], BF16, tag="gt")
                        nc.vector.tensor_copy(out=gt[:], in_=t1[:])
                        gts[bi] = gt
                    # ---- phase 3: W matmul + relu into pool tile + h store
                    if "psi" in os.environ.get("GCN_SKIP", ""):
                        continue
                    for bi, b in enumerate(blks):
                        gt = gts[bi]
                        psi_ps = psipsp.tile([P, NS * P], F32, tag="psip")
                        if li == 0:
                            for s_in in range(NS1):
                                for v in range(2):
                                    nc.tensor.matmul(
                                        psi_ps[:, (2 * s_in + v) * P:
                                               (2 * s_in + v + 1) * P],
                                        lhsT=wzt[:, v * P:(v + 1) * P],
                                        rhs=gt[:, s_in * P:(s_in + 1) * P],
                                        start=True, stop=True,
                                    )
                        else:
                            wv = 1 + li
                            for s in range(NS):
                                nc.tensor.matmul(
                                    psi_ps[:, s * P:(s + 1) * P],
                                    lhsT=wzt[:, wv * P:(wv + 1) * P],
                                    rhs=gt[:, s * P:(s + 1) * P],
                                    start=True, stop=True,
                                )
                        # relu + bias straight into the group pooling tile
                        gwk = len(blks) * P
                        dst_view = psi_grp[:].rearrange(
                            "p (s n) -> p s n", n=gwk)[:, :, bi * P:(bi + 1) * P]
                        nc.scalar.activation(
                            dst_view,
                            psi_ps[:].rearrange("p (s n) -> p s n", s=NS),
                            mybir.ActivationFunctionType.Relu,
                            bias=bct[:, li:li + 1],
                        )
                        if li < 2 and nlayers > li + 1 and \
                                "t2" not in os.environ.get("GCN_SKIP", ""):
                            psb = workp.tile([P, NS * P], BF16, tag="psb")
                            nc.scalar.activation(
                                psb[:], psi_ps[:],
                                mybir.ActivationFunctionType.Relu,
                                bias=bct[:, li:li + 1],
                            )
                            t2 = t2psp.tile([P, NS * P], BF16, tag="t2")
                            for s in range(NS):
                                nc.tensor.transpose(
                                    t2[:, s * P:(s + 1) * P],
                                    psb[:, s * P:(s + 1) * P], idb[:])
                            hbf = workp.tile([P, CH], BF16, tag="hbf")
                            nc.vector.tensor_copy(out=hbf[:], in_=t2[:])
                            nc.sync.dma_start(
                                out=h_mine[li][b * P:(b + 1) * P, :], in_=hbf[:])

                    # ---- pooling for this group
                    if "pool" not in os.environ.get("GCN_SKIP", ""):
                        gw = len(blks) * P
                        for s in range(NS):
                            base = s * gw
                            for (n0, n1, gl, ft) in pool_pieces[gi]:
                                seg = psi_grp[:, base + n0: base + n1]
                                slot = slice(s * cfg.GPC + gl, s * cfg.GPC + gl + 1)
                                if ft:
                                    nc.vector.reduce_max(
                                        out=lmax[:, slot], in_=seg,
                                        axis=mybir.AxisListType.X)
                                    nc.vector.reduce_sum(
                                        out=lsum[:, slot], in_=seg,
                                        axis=mybir.AxisListType.X)
                                else:
                                    tm = workp.tile([P, 2], F32, tag="ptmp")
                                    nc.vector.reduce_max(out=tm[:, 0:1], in_=seg,
                                                         axis=mybir.AxisListType.X)
                                    nc.vector.reduce_sum(out=tm[:, 1:2], in_=seg,
                                                         axis=mybir.AxisListType.X)
                                    nc.vector.tensor_tensor(
                                        out=lmax[:, slot], in0=lmax[:, slot],
                                        in1=tm[:, 0:1], op=mybir.AluOpType.max)
                                    nc.vector.tensor_add(
                                        out=lsum[:, slot], in0=lsum[:, slot],
                                        in1=tm[:, 1:2])

                    # ---- AllGather: schedule per completed segment, but
                    # emit the trigger ~2 groups later so its h_mine deps are
                    # already satisfied and it never stalls the gather stream
                    if li < 2 and nlayers > li + 1 and gi in seg_last_group:
                        si = seg_last_group.index(gi)
                        dgi = min(gi + 2, ngroups - 1)
                        due = (li, dgi) if dgi > gi else (li + 1, 0)
                        pending_ags.append((due, li, si))

                # ---- layer end: accumulate pools
                if "pool" in os.environ.get("GCN_SKIP", ""):
                    pass
                elif li == 0:
                    nc.vector.tensor_copy(out=fmax[:], in_=lmax[:])
                    nc.vector.tensor_copy(out=fsum[:], in_=lsum[:])
                else:
                    nc.vector.tensor_add(out=fmax[:], in0=fmax[:], in1=lmax[:])
                    nc.vector.tensor_add(out=fsum[:], in0=fsum[:], in1=lsum[:])

            for _li in range(nlayers):
                layer(_li)

            # mean = sum / GRAPH
            nc.vector.tensor_scalar_mul(fsum[:], fsum[:],
                                        float(np.float32(1.0 / cfg.GRAPH)))
            osb = workp.tile([P, 2 * NS * cfg.GPC], F32, tag="osb")
            nc.vector.tensor_copy(out=osb[:, :NS * cfg.GPC], in_=fmax[:])
            nc.vector.tensor_copy(out=osb[:, NS * cfg.GPC:], in_=fsum[:])
            nc.sync.dma_start(out=out[:], in_=osb[:])

    nc.compile()
    return nc


def unshard(cfg, results):
    """[NCORES][128, 2*NS*GPC] -> [B, 2*COUT, T] float32."""
    B, T, COUT, NS, GPC = cfg.B, cfg.T, cfg.COUT, cfg.NS, cfg.GPC
    out = np.zeros((B, 2 * COUT, T), np.float32)
    for c in range(cfg.NCORES):
        V = results[c]["out"]
        for gl in range(cfg.gpc[c]):
            g = cfg.goff[c] + gl
            for s in range(NS):
                for half in range(2):
                    t_ = 2 * s + half
                    co = np.arange(COUT)
                    pp = half * COUT + co
                    out[g, co, t_] = V[pp, s * GPC + gl]
                    out[g, COUT + co, t_] = V[pp, NS * GPC + s * GPC + gl]
    return out


_CACHE = {}


def kernel(**inputs):
    cfg = Cfg()
    common, per_core, meta = preprocess(
        cfg, inputs["x"], inputs["edge_index"], inputs["batch"],
        inputs["W1"], inputs["b1"], inputs["W2"], inputs["b2"],
        inputs["W3"], inputs["b3"])
    key = (meta["KL"], meta["KH"], meta["NCALLS"], meta["KT"].tobytes())
    if key not in _CACHE:
        _CACHE[key] = build(cfg, meta)
    nc = _CACHE[key]
    in_maps = []
    for c in range(cfg.NCORES):
        m = dict(common)
        m["o23"] = per_core[c]["o23"]
        m["gidx"] = per_core[c]["gidx"]
        in_maps.append(m)
    res = run_bass_kernel_spmd(nc, in_maps, list(range(cfg.NCORES)))
    return unshard(cfg, res.results)


# revision 16
# speedup vs baseline: 1.1483x; 1.0876x over previous
"""Trainium2 Bass kernel for nn_GCNLayer (3-layer GCN + max/mean pooling, T temporal slices).

Self-contained: hardcodes the problem shapes (N=50000, E=800000, B=250, T=8,
CIN=32, COUT=64) and distributes over 8 NeuronCores by graph/dst-node range.

Algorithm per layer (S = sym-normalized adjacency incl. self-loops):
    H_out = relu((S @ H_in) @ W + b)
computed edge-parallel per core:
  - dma_gather of H_in[src] rows (bf16, T-packed rows), round-robined across
    the 4 SWDGE queues so descriptor generation runs on all 4 Q7 core pairs
  - scatter-add via one-hot matmul: lhsT = O (128 edges x 128 dst slots,
    norm values baked in), rhs = gathered messages, PSUM-accumulated per
    128-node dst block
  - PE transpose (bf16) -> fused block-diagonal W matmul -> relu+bias on ACT
    written straight into the pooling group tile
  - pooling (max + mean over each graph's 200 nodes) via free-dim reduces
  - transpose back, store bf16 H to DRAM; AllGather across the 8 cores in
    4 segment chunks so most of the collective overlaps compute
"""

import os
import numpy as np
import ml_dtypes

import concourse.bass as bass
import concourse.mybir as mybir
from concourse import bacc, tile
from concourse.bass_utils import run_bass_kernel_spmd

F32 = mybir.dt.float32
BF16 = mybir.dt.bfloat16
I16 = mybir.dt.int16
P = 128
NQUEUES = 4
SEG_BLOCKS = [16, 16, 8, 8, 2]      # 50 blocks split into AllGather segments
GRP = 4                              # blocks per processing group


class Cfg:
    def __init__(self, N=50000, E=800000, B=250, T=8, CIN=32, COUT=64,
                 NCORES=8, GRAPH=200):
        self.N, self.E, self.B, self.T = N, E, B, T
        self.CIN, self.COUT, self.NCORES, self.GRAPH = CIN, COUT, NCORES, GRAPH
        base, rem = divmod(B, NCORES)
        self.gpc = [base + (1 if c < rem else 0) for c in range(NCORES)]
        self.GPC = max(self.gpc)                      # uniform per-core graph slots
        self.NPC = self.GPC * GRAPH                   # padded nodes per core
        assert self.NPC % P == 0
        self.NBLK = self.NPC // P                     # dst blocks per core
        assert sum(SEG_BLOCKS) == self.NBLK
        self.NPAD = self.NPC * NCORES                 # padded global node count
        # segment row ranges (local, per core) and global bases
        self.seg_rows = [b * P for b in SEG_BLOCKS]
        self.seg_lo_local = np.concatenate([[0], np.cumsum(self.seg_rows)]).astype(np.int64)
        self.seg_lo_global = self.seg_lo_local * NCORES
        self.HALF = int((self.seg_rows[0] + self.seg_rows[1]) * NCORES)
        assert self.HALF <= 32768
        assert self.NPAD - self.HALF <= 32768
        self.CH1 = CIN * T                            # layer-1 feature row
        self.CH = COUT * T                            # layer-2/3 feature row
        assert self.CH % P == 0
        self.NS = self.CH // P                        # psi partition tiles (t-pairs)
        # graph id offset per core
        self.goff = np.concatenate([[0], np.cumsum(self.gpc)]).astype(np.int64)

    def local2seg(self, local):
        """core-local padded row -> segment index"""
        return np.searchsorted(self.seg_lo_local, local, side="right") - 1


def _groups():
    """group block ranges; groups never span AllGather segments."""
    groups = []
    seg_last_group = []
    b0 = 0
    for sb in SEG_BLOCKS:
        end = b0 + sb
        while b0 < end:
            groups.append(list(range(b0, min(b0 + GRP, end))))
            b0 += GRP if b0 + GRP <= end else (end - b0)
        seg_last_group.append(len(groups) - 1)
    return groups, seg_last_group


def preprocess(cfg, x, edge_index, batch, W1, b1, W2, b2, W3, b3):
    """Build all per-core device inputs. Returns (common_inputs, per_core_inputs, meta)."""
    N, E, T, CIN, COUT = cfg.N, cfg.E, cfg.T, cfg.CIN, cfg.COUT
    src = np.asarray(edge_index[0], np.int64)
    dst = np.asarray(edge_index[1], np.int64)

    # degrees incl self-loops, matching the reference
    deg = np.bincount(dst, minlength=N).astype(np.float32) + 1.0
    dinv = (1.0 / np.sqrt(deg)).astype(np.float32)

    batch = np.asarray(batch, np.int64)
    g2c = np.zeros(cfg.B, np.int64)
    for c in range(cfg.NCORES):
        g2c[cfg.goff[c]:cfg.goff[c + 1]] = c
    node_core = g2c[batch]
    first_node_of_core = np.array([cfg.goff[c] * cfg.GRAPH for c in range(cfg.NCORES)],
                                  np.int64)
    local_n = np.arange(N) - first_node_of_core[node_core]
    # padded id: segment-major, then core-major within segment
    seg = cfg.local2seg(local_n)
    seg_rows = np.asarray(cfg.seg_rows, np.int64)
    pad_id = (cfg.seg_lo_global[seg] + node_core * seg_rows[seg]
              + (local_n - cfg.seg_lo_local[seg]))

    srcp = pad_id[src]
    dstc = node_core[dst]
    dstl = local_n[dst]

    # X permuted to [NPAD, T*CIN] (t-major rows), bf16
    Xp = np.zeros((cfg.NPAD, cfg.CH1), np.float32)
    xm = np.moveaxis(np.asarray(x, np.float32), 2, 1).reshape(N, T * CIN)
    Xp[pad_id] = xm

    groups, seg_last_group = _groups()

    # per-core edge bucketing (self-loops included as edges)
    per_core = []
    # per-(block, half) chunk count: max over cores (keeps SPMD program
    # uniform while dropping the global-max padding)
    KT = np.ones((cfg.NBLK, 2), np.int64)
    for c in range(cfg.NCORES):
        m = dstc == c
        es, ed = srcp[m], dstl[m]
        nv = dinv[src[m]] * dinv[dst[m]]
        ln = np.where(node_core == np.int64(c))[0]
        es = np.concatenate([es, pad_id[ln]])
        ed = np.concatenate([ed, local_n[ln]])
        nv = np.concatenate([nv, dinv[ln] * dinv[ln]])
        blk = ed // P
        half = (es >= cfg.HALF).astype(np.int64)
        order = np.lexsort((es, half, blk))
        es, ed, nv, blk, half = (a[order] for a in (es, ed, nv, blk, half))
        per_core.append((es, ed, nv, blk, half))
        for b in range(cfg.NBLK):
            mb = blk == b
            nlo = int(((half == 0) & mb).sum())
            nhi = int(((half == 1) & mb).sum())
            KT[b, 0] = max(KT[b, 0], -(-nlo // P))
            KT[b, 1] = max(KT[b, 1], -(-nhi // P))
    KL, KH = int(KT[:, 0].max()), int(KT[:, 1].max())

    def call_splits(nch):
        out, pos = [], 0
        while pos < nch:
            k = min(8, nch - pos)
            out.append((pos, k))
            pos += k
        return out

    # call table: (half, group_idx, chunk0_in_group, nchunks)
    calls = []
    for h in (0, 1):
        for gi, blks in enumerate(groups):
            nch = int(sum(KT[b, h] for b in blks))
            for pos, k in call_splits(nch):
                calls.append((h, gi, pos, k))
    NCALLS = len(calls)

    chunk_map = {}
    for ci, (h, gi, pos, k) in enumerate(calls):
        for j in range(k):
            chunk_map[(h, gi, pos + j)] = (ci, j)

    # chunk index within a group: chunk q of block b (half h) sits at
    # offset sum(KT[b', h] for earlier blocks in the group) + q
    def chunk_in_group(h, blks, bi, q):
        return int(sum(KT[b, h] for b in blks[:bi]) + q)

    # build per-core O (bf16) + gather idx + host-pre-gathered layer-0
    # messages xg (layer 0's gather sources are the static input x, so the
    # permutation is done on the host and streamed as sequential DMA)
    per_core_inputs = []
    for c in range(cfg.NCORES):
        es, ed, nv, blk, half = per_core[c]
        O = np.zeros((NCALLS, P, 8 * P), np.float32)
        idx = np.zeros((P, NCALLS * 64), np.int16)
        xg = np.zeros((NCALLS, P, 8 * cfg.CH1), np.float32)
        for gi, blks in enumerate(groups):
            for h in (0, 1):
                for bi, b in enumerate(blks):
                    K = int(KT[b, h])
                    m = (blk == b) & (half == h)
                    e_s, e_d, e_n = es[m], ed[m], nv[m]
                    n_e = len(e_s)
                    assert n_e <= K * P
                    for k in range(K):
                        ci, j = chunk_map[(h, gi, chunk_in_group(h, blks, bi, k))]
                        lo, hi = k * P, min((k + 1) * P, n_e)
                        cnt = max(0, hi - lo)
                        vals = np.zeros(P, np.int64)
                        if cnt:
                            vals[:cnt] = e_s[lo:hi] - (cfg.HALF if h else 0)
                        ii = j * P + np.arange(P)
                        idx[ii % 16, ci * 64 + ii // 16] = vals.astype(np.int16)
                        if cnt:
                            rows = np.arange(cnt)
                            cols = j * P + (e_d[lo:hi] - b * P)
                            O[ci, rows, cols] = e_n[lo:hi]
                            xg[ci, :cnt, j * cfg.CH1:(j + 1) * cfg.CH1] = \
                                Xp[e_s[lo:hi]]
        idx[16:] = np.tile(idx[:16], (7, 1))
        per_core_inputs.append({
            "o23": O.astype(ml_dtypes.bfloat16),
            "gidx": idx,
            "xg": xg.astype(ml_dtypes.bfloat16),
        })

    # pooling piece table per group
    pool_pieces = []
    seen = set()
    for gi, blks in enumerate(groups):
        n0g = blks[0] * P
        n1g = (blks[-1] + 1) * P
        pieces = []
        n = n0g
        while n < n1g:
            gl = n // cfg.GRAPH
            nend = min((gl + 1) * cfg.GRAPH, n1g)
            ft = gl not in seen
            seen.add(gl)
            pieces.append((n - n0g, nend - n0g, gl, ft))
            n = nend
        pool_pieces.append(pieces)

    # fused block-diagonal psi weights: [4, 128, 128]
    #   0: layer0 variant A (q=0,1 -> s_out even), 1: layer0 variant B (q=2,3)
    #   2: layer1, 3: layer2
    wz = np.zeros((4, P, P), np.float32)
    W1, W2, W3 = (np.asarray(w, np.float32) for w in (W1, W2, W3))
    wz[0, 0:32, 0:64] = W1
    wz[0, 32:64, 64:128] = W1
    wz[1, 64:96, 0:64] = W1
    wz[1, 96:128, 64:128] = W1
    wz[2, 0:64, 0:64] = W2
    wz[2, 64:128, 64:128] = W2
    wz[3, 0:64, 0:64] = W3
    wz[3, 64:128, 64:128] = W3

    bias_col = np.zeros((P, 3), np.float32)
    for i, b in enumerate((b1, b2, b3)):
        bias_col[:, i] = np.tile(np.asarray(b, np.float32), P // COUT)

    ident = np.eye(P, dtype=np.float32)
    common = {
        "wz": wz.astype(ml_dtypes.bfloat16),
        "biascol": bias_col,
        "id_bf": ident.astype(ml_dtypes.bfloat16),
    }
    meta = dict(KL=KL, KH=KH, KT=KT, calls=calls, chunk_map=chunk_map,
                groups=groups, seg_last_group=seg_last_group,
                pool_pieces=pool_pieces, NCALLS=NCALLS)
    return common, per_core_inputs, meta


def build(cfg, meta):
    """Construct the Bass/Tile SPMD program."""
    KL, KH, calls, chunk_map = meta["KL"], meta["KH"], meta["calls"], meta["chunk_map"]
    KT = meta["KT"]

    def chunk_in_group(h, blks, bi, q):
        return int(sum(KT[b, h] for b in blks[:bi]) + q)
    groups, pool_pieces, NCALLS = meta["groups"], meta["pool_pieces"], meta["NCALLS"]
    seg_last_group = meta["seg_last_group"]
    NS, CH, CH1, T, COUT = cfg.NS, cfg.CH, cfg.CH1, cfg.T, cfg.COUT
    NS1 = max(CH1 // P, 1)
    CIN = cfg.CIN

    nc = bacc.Bacc("TRN2", target_bir_lowering=False, debug=False,
                   num_devices=cfg.NCORES, num_swdge_queues=NQUEUES)

    xg = nc.dram_tensor("xg", [NCALLS, P, 8 * CH1], BF16, kind="ExternalInput")
    o23 = nc.dram_tensor("o23", [NCALLS, P, 8 * P], BF16, kind="ExternalInput")
    gidx = nc.dram_tensor("gidx", [P, NCALLS * 64], I16, kind="ExternalInput")
    wz_d = nc.dram_tensor("wz", [4, P, P], BF16, kind="ExternalInput")
    biascol = nc.dram_tensor("biascol", [P, 3], F32, kind="ExternalInput")
    id_bf = nc.dram_tensor("id_bf", [P, P], BF16, kind="ExternalInput")
    out = nc.dram_tensor("out", [P, 2 * NS * cfg.GPC], F32, kind="ExternalOutput")

    rg = [list(range(cfg.NCORES))]
    qn = [0]  # round-robin SWDGE queue counter

    with tile.TileContext(nc) as tc:
        with (
            tc.tile_pool(name="const", bufs=1) as constp,
            tc.tile_pool(name="msg", bufs=10) as msgp,
            tc.tile_pool(name="msgh", bufs=4) as msghp,
            tc.tile_pool(name="otile", bufs=8) as otp,
            tc.tile_pool(name="oth", bufs=6) as othp,
            tc.tile_pool(name="work", bufs=4) as workp,
            tc.tile_pool(name="psig", bufs=2) as psigp,
            tc.tile_pool(name="pool", bufs=1) as poolp,
            tc.tile_pool(name="gps", bufs=3, space="PSUM") as gpsp,
            tc.tile_pool(name="t1ps", bufs=2, space="PSUM") as t1psp,
            tc.tile_pool(name="psips", bufs=2, space="PSUM") as psipsp,
            tc.tile_pool(name="t2ps", bufs=1, space="PSUM") as t2psp,
            tc.tile_pool(name="dram", bufs=1, space="DRAM") as dramp,
        ):
            # ---- constants into SBUF
            idx_sb = constp.tile([P, NCALLS * 64], I16)
            nc.sync.dma_start(out=idx_sb[:], in_=gidx[:])
            wzt = constp.tile([P, 4 * P], BF16, tag="wzt")
            nc.sync.dma_start(
                out=wzt[:].rearrange("p (i m) -> p i m", i=4),
                in_=wz_d.ap().rearrange("i p m -> p i m"))
            bct = constp.tile([P, 3], F32)
            nc.sync.dma_start(out=bct[:], in_=biascol[:])
            idb = constp.tile([P, P], BF16)
            nc.sync.dma_start(out=idb[:], in_=id_bf[:])

            # ---- pool accumulators
            lmax = poolp.tile([P, NS * cfg.GPC], F32, tag="lmax")
            lsum = poolp.tile([P, NS * cfg.GPC], F32, tag="lsum")
            fmax = poolp.tile([P, NS * cfg.GPC], F32, tag="fmax")
            fsum = poolp.tile([P, NS * cfg.GPC], F32, tag="fsum")
            for _t in (lmax, lsum, fmax, fsum):
                nc.vector.memset(_t[:], 0.0)

            # ---- DRAM intermediates
            h_mine = []
            h_full = []
            for i in range(2):
                hm = dramp.tile([cfg.NPC, CH], BF16, tag=f"hm{i}")
                h_mine.append(hm)
                hf = dramp.tile([cfg.NPAD, CH], BF16, tag=f"hf{i}")
                h_full.append(hf)

            nlayers = int(os.environ.get("GCN_LAYERS", "3"))
            pending_ags = []

            def emit_due_ags(li, gi):
                for ent in list(pending_ags):
                    (dli, dgi), ali, si = ent
                    if (dli, dgi) != (li, gi):
                        continue
                    pending_ags.remove(ent)
                    llo = int(cfg.seg_lo_local[si])
                    lhi = int(cfg.seg_lo_local[si + 1])
                    glo = int(cfg.seg_lo_global[si])
                    ghi = int(cfg.seg_lo_global[si + 1])
                    nc.gpsimd.collective_compute(
                        "AllGather", mybir.AluOpType.bypass,
                        replica_groups=rg,
                        ins=[h_mine[ali][llo:lhi, :]],
                        outs=[h_full[ali][glo:ghi, :]],
                    )

            def layer(li):
                ch_in = CH1 if li == 0 else CH
                ns_in = NS1 if li == 0 else NS
                if li == 0:
                    src_lo = src_hi = None
                else:
                    hsrc = h_full[li - 1]
                    src_lo = hsrc[:cfg.HALF, :]
                    src_hi = hsrc[cfg.HALF:cfg.NPAD, :]

                lo_calls, hi_calls = {}, {}
                for ci, (h, gi, pos, k) in enumerate(calls):
                    (lo_calls if h == 0 else hi_calls).setdefault(gi, []).append(
                        (ci, h, pos, k))

                gtiles = {}

                def emit_calls(cl):
                    for ci, h, pos, k in cl:
                        ni = k * P
                        g = (msgp if h == 0 else msghp).tile(
                            [P, 8 * ch_in], BF16, tag=f"m{h}")
                        if li == 0:
                            # host-pre-gathered: plain sequential DMA
                            nc.sync.dma_start(
                                out=g[:, :k * ch_in], in_=xg[ci, :, :k * ch_in])
                        else:
                            nc.gpsimd.dma_gather(
                                out_ap=g[:, :k * ch_in].rearrange(
                                    "p (c e) -> p c e", e=ch_in),
                                in_ap=(src_lo if h == 0 else src_hi),
                                idxs_ap=idx_sb[:, ci * 64:
                                               ci * 64 + max(ni // 16, 1)],
                                num_idxs=ni,
                                num_idxs_reg=ni,
                                elem_size=ch_in,
                                queue_num=qn[0] % NQUEUES,
                            )
                            qn[0] += 1
                        ot = (otp if h == 0 else othp).tile(
                            [P, 8 * P], BF16, tag=f"oo{h}")
                        nc.sync.dma_start(out=ot[:, :k * P], in_=o23[ci, :, :k * P])
                        gtiles[ci] = (g, ot)

                ngroups = len(groups)
                for gi, blks in enumerate(groups):
                    # lo-half gathers run one group ahead; hi-half gathers of
                    # this group go last so a pending tail AllGather (high
                    # segments) doesn't head-of-line-block the stream.
                    if gi == 0:
                        emit_calls(lo_calls[0])
                    if gi + 1 < ngroups:
                        emit_calls(lo_calls[gi + 1])
                    emit_due_ags(li, gi)
                    emit_calls(hi_calls[gi])

                    psi_grp = psigp.tile([P, NS * len(blks) * P], F32, tag="psig")
                    # ---- phase 1: scatter matmuls + PSUM->SBUF bf16 cast
                    gbfbs = {}
                    for bi, b in enumerate(blks):
                        gps = gpsp.tile([P, ch_in], F32, tag="gps")
                        nmm = int(KT[b, 0] + KT[b, 1])
                        mm = 0
                        for h in (0, 1):
                            K = int(KT[b, h])
                            for k in range(K):
                                ci, j = chunk_map[(h, gi, chunk_in_group(h, blks, bi, k))]
                                g, ot = gtiles[ci]
                                nc.tensor.matmul(
                                    gps[:],
                                    lhsT=ot[:, j * P:(j + 1) * P],
                                    rhs=g[:, j * ch_in:(j + 1) * ch_in],
                                    start=(mm == 0), stop=(mm == nmm - 1),
                                )
                                mm += 1
                        gbfb = workp.tile([P, ch_in], BF16, tag="gbfb")
                        nc.scalar.activation(
                            gbfb[:], gps[:], mybir.ActivationFunctionType.Copy)
                        gbfbs[bi] = gbfb
                    # ---- phase 2: transpose to channel-major
                    gts = {}
                    for bi, b in enumerate(blks):
                        t1 = t1psp.tile([P, ns_in * P], BF16, tag="t1")
                        for s in range(ns_in):
                            nc.tensor.transpose(
                                t1[:, s * P:(s + 1) * P],
                                gbfbs[bi][:, s * P:(s + 1) * P], idb[:])
                        gt = workp.tile([P, ns_in * P], BF16, tag="gt")
                        nc.vector.tensor_copy(out=gt[:], in_=t1[:])
                        gts[bi] = gt
                    # ---- phase 3: W matmul + relu into pool tile + h store
                    if "psi" in os.environ.get("GCN_SKIP", ""):
                        continue
                    for bi, b in enumerate(blks):
                        gt = gts[bi]
                        psi_ps = psipsp.tile([P, NS * P], F32, tag="psip")
                        if li == 0:
                            for s_in in range(NS1):
                                for v in range(2):
                                    nc.tensor.matmul(
                                        psi_ps[:, (2 * s_in + v) * P:
                                               (2 * s_in + v + 1) * P],
                                        lhsT=wzt[:, v * P:(v + 1) * P],
                                        rhs=gt[:, s_in * P:(s_in + 1) * P],
                                        start=True, stop=True,
                                    )
                        else:
                            wv = 1 + li
                            for s in range(NS):
                                nc.tensor.matmul(
                                    psi_ps[:, s * P:(s + 1) * P],
                                    lhsT=wzt[:, wv * P:(wv + 1) * P],
                                    rhs=gt[:, s * P:(s + 1) * P],
                                    start=True, stop=True,
                                )
                        # relu + bias straight into the group pooling tile
                        gwk = len(blks) * P
                        dst_view = psi_grp[:].rearrange(
                            "p (s n) -> p s n", n=gwk)[:, :, bi * P:(bi + 1) * P]
                        nc.scalar.activation(
                            dst_view,
                            psi_ps[:].rearrange("p (s n) -> p s n", s=NS),
                            mybir.ActivationFunctionType.Relu,
                            bias=bct[:, li:li + 1],
                        )
                        if li < 2 and nlayers > li + 1 and \
                                "t2" not in os.environ.get("GCN_SKIP", ""):
                            psb = workp.tile([P, NS * P], BF16, tag="psb")
                            nc.scalar.activation(
                                psb[:], psi_ps[:],
                                mybir.ActivationFunctionType.Relu,
                                bias=bct[:, li:li + 1],
                            )
                            t2 = t2psp.tile([P, NS * P], BF16, tag="t2")
                            for s in range(NS):
                                nc.tensor.transpose(
                                    t2[:, s * P:(s + 1) * P],
                                    psb[:, s * P:(s + 1) * P], idb[:])
                            hbf = workp.tile([P, CH], BF16, tag="hbf")
                            nc.vector.tensor_copy(out=hbf[:], in_=t2[:])
                            nc.sync.dma_start(
                                out=h_mine[li][b * P:(b + 1) * P, :], in_=hbf[:])

                    # ---- pooling for this group
                    if "pool" not in os.environ.get("GCN_SKIP", ""):
                        gw = len(blks) * P
                        for s in range(NS):
                            base = s * gw
                            for (n0, n1, gl, ft) in pool_pieces[gi]:
                                seg = psi_grp[:, base + n0: base + n1]
                                slot = slice(s * cfg.GPC + gl, s * cfg.GPC + gl + 1)
                                if ft:
                                    nc.vector.reduce_max(
                                        out=lmax[:, slot], in_=seg,
                                        axis=mybir.AxisListType.X)
                                    nc.vector.reduce_sum(
                                        out=lsum[:, slot], in_=seg,
                                        axis=mybir.AxisListType.X)
                                else:
                                    tm = workp.tile([P, 2], F32, tag="ptmp")
                                    nc.vector.reduce_max(out=tm[:, 0:1], in_=seg,
                                                         axis=mybir.AxisListType.X)
                                    nc.vector.reduce_sum(out=tm[:, 1:2], in_=seg,
                                                         axis=mybir.AxisListType.X)
                                    nc.vector.tensor_tensor(
                                        out=lmax[:, slot], in0=lmax[:, slot],
                                        in1=tm[:, 0:1], op=mybir.AluOpType.max)
                                    nc.vector.tensor_add(
                                        out=lsum[:, slot], in0=lsum[:, slot],
                                        in1=tm[:, 1:2])

                    # ---- AllGather: schedule per completed segment, but
                    # emit the trigger ~2 groups later so its h_mine deps are
                    # already satisfied and it never stalls the gather stream
                    if li < 2 and nlayers > li + 1 and gi in seg_last_group:
                        si = seg_last_group.index(gi)
                        dgi = min(gi + 2, ngroups - 1)
                        due = (li, dgi) if dgi > gi else (li + 1, 0)
                        pending_ags.append((due, li, si))

                # ---- layer end: accumulate pools
                if "pool" in os.environ.get("GCN_SKIP", ""):
                    pass
                elif li == 0:
                    nc.vector.tensor_copy(out=fmax[:], in_=lmax[:])
                    nc.vector.tensor_copy(out=fsum[:], in_=lsum[:])
                else:
                    nc.vector.tensor_add(out=fmax[:], in0=fmax[:], in1=lmax[:])
                    nc.vector.tensor_add(out=fsum[:], in0=fsum[:], in1=lsum[:])

            for _li in range(nlayers):
                layer(_li)

            # mean = sum / GRAPH
            nc.vector.tensor_scalar_mul(fsum[:], fsum[:],
                                        float(np.float32(1.0 / cfg.GRAPH)))
            osb = workp.tile([P, 2 * NS * cfg.GPC], F32, tag="osb")
            nc.vector.tensor_copy(out=osb[:, :NS * cfg.GPC], in_=fmax[:])
            nc.vector.tensor_copy(out=osb[:, NS * cfg.GPC:], in_=fsum[:])
            nc.sync.dma_start(out=out[:], in_=osb[:])

    nc.compile()
    return nc


def unshard(cfg, results):
    """[NCORES][128, 2*NS*GPC] -> [B, 2*COUT, T] float32."""
    B, T, COUT, NS, GPC = cfg.B, cfg.T, cfg.COUT, cfg.NS, cfg.GPC
    out = np.zeros((B, 2 * COUT, T), np.float32)
    for c in range(cfg.NCORES):
        V = results[c]["out"]
        for gl in range(cfg.gpc[c]):
            g = cfg.goff[c] + gl
            for s in range(NS):
                for half in range(2):
                    t_ = 2 * s + half
                    co = np.arange(COUT)
                    pp = half * COUT + co
                    out[g, co, t_] = V[pp, s * GPC + gl]
                    out[g, COUT + co, t_] = V[pp, NS * GPC + s * GPC + gl]
    return out


_CACHE = {}


def kernel(**inputs):
    cfg = Cfg()
    common, per_core, meta = preprocess(
        cfg, inputs["x"], inputs["edge_index"], inputs["batch"],
        inputs["W1"], inputs["b1"], inputs["W2"], inputs["b2"],
        inputs["W3"], inputs["b3"])
    key = (meta["KL"], meta["KH"], meta["NCALLS"], meta["KT"].tobytes())
    if key not in _CACHE:
        _CACHE[key] = build(cfg, meta)
    nc = _CACHE[key]
    in_maps = []
    for c in range(cfg.NCORES):
        m = dict(common)
        m["o23"] = per_core[c]["o23"]
        m["gidx"] = per_core[c]["gidx"]
        m["xg"] = per_core[c]["xg"]
        in_maps.append(m)
    res = run_bass_kernel_spmd(nc, in_maps, list(range(cfg.NCORES)))
    return unshard(cfg, res.results)
